# revision 43
# baseline (speedup 1.0000x reference)
"""TopK autoencoder (B=4096, D=1024, F=32768, K=64) on 8 Trainium2 NeuronCores.

Strategy: data-parallel over batch (512 rows/core). Per core, per 128-row tile:
  A) fp16-split encoder matmul (PE) accumulated in PSUM fp32; relu on ACT into
     an fp32 staging buffer spilled to DRAM in 1MB blocks; per-group (32) maxima
     reduced on DVE straight from PSUM (pre-relu max == relu'd max for the
     groups that matter).
  B) exact top-K: group maxima packed as (fp16-bits << 16 | group-id); top-NP
     groups via max8/match_replace rounds; ONE batched indirect DMA gathers all
     NP*32 fp32 candidates; candidates packed with feature tags; exact K-th
     value from destructive max rounds; threshold-mask then extract (value,
     index) pairs.
  C) decode: batched indirect gather of selected W_dec rows (fp16), accumulate
     w_k * row_k on the PE via diagonal-matrix matmuls into PSUM.
b_dec is handled exactly on the host (x - b_dec, + b_dec at the end); a
nonzero b_enc is folded in as an extra contraction tile (zero here).
"""
import sys
sys.path.insert(0, '/opt/trn_rl_repo')
import numpy as np
import concourse.bass as bass
import concourse.mybir as mybir
from concourse import bacc
from concourse.bass import ts, ds
from concourse.tile import TileContext
from concourse.masks import make_identity
from concourse.bass_utils import run_bass_kernel_spmd

f32 = mybir.dt.float32
f16 = mybir.dt.float16
bf16 = mybir.dt.bfloat16
u16 = mybir.dt.uint16
u32 = mybir.dt.uint32
i32 = mybir.dt.int32
i16 = mybir.dt.int16
Alu = mybir.AluOpType
Act = mybir.ActivationFunctionType

B, D, F, K = 4096, 1024, 32768, 64
N_CORES = 8
GE = 32      # group size
NP = 72      # candidate groups per row
PB = 8       # decode gather block
NSPLIT = 2   # matmul passes: 3 = exact-fp32 selection; 2 = x-split only

_CACHE = {}
ABLATE = None   # None | "a" | "bg" | "b"  (timing experiments only)


def build(DX, DO, FF, BL, n_cores=N_CORES, reps=1, nsplit=NSPLIT):
    """Per-core kernel. DX: contraction dim (may include bias tile), DO: output dim."""
    KT = DX // 128
    NT = BL // 128
    FC = 512
    NFC = FF // FC
    NG = FF // GE
    GPC = FC // GE    # groups per chunk
    CB = 4            # chunks per spill block
    WS = 2 if nsplit == 3 else 1   # number of W split planes
    XS = nsplit                    # number of x planes
    # pass structure: (x plane, w plane)
    if nsplit == 3:
        PASSES = ((0, 0), (1, 1), (2, 0))
    else:
        PASSES = ((0, 0), (1, 0))

    nc = bacc.Bacc("TRN2", target_bir_lowering=False, debug=False, num_devices=n_cores)
    xt = nc.dram_tensor("xt", [XS, DX, BL], f16, kind="ExternalInput")
    # W blocked per chunk: [NFC, 128, WS, KT, FC] so each chunk load is
    # contiguous 16KB per partition (line-rate DMA)
    wencT = nc.dram_tensor("wencT", [NFC, 128, WS, KT, FC], f16, kind="ExternalInput")
    wdecT = nc.dram_tensor("wdecT", [FF, DO], f16, kind="ExternalInput")
    out = nc.dram_tensor("out", [BL, DO], f32, kind="ExternalOutput")

    wencT_r = wencT.ap().rearrange("n p s k f -> p n s k f")
    xt_r = xt.ap().rearrange("s (k p) b -> p s k b", p=128)

    with TileContext(nc) as tc:
        with (
            tc.tile_pool(name="dram", bufs=4, space="DRAM") as dpool,
            tc.tile_pool(name="xt_sb", bufs=1) as xpool,
            tc.tile_pool(name="const", bufs=1) as kpool,
            tc.tile_pool(name="wenc", bufs=2) as wpool,
            tc.tile_pool(name="apsum", bufs=4, space="PSUM") as apsum,
            tc.tile_pool(name="abuf", bufs=2) as apool,
            tc.tile_pool(name="dg", bufs=8) as dgpool,
            tc.tile_pool(name="gbuf", bufs=4) as gpool,
            tc.tile_pool(name="cand", bufs=1) as cpool,
            tc.tile_pool(name="pack", bufs=1) as ppool,
            tc.tile_pool(name="small", bufs=2) as spool,
            tc.tile_pool(name="wdecg", bufs=2) as wgpool,
            tc.tile_pool(name="dpsum", bufs=1, space="PSUM") as dpsum,
            tc.tile_pool(name="tpsum", bufs=1, space="PSUM") as tpsum,
            tc.tile_pool(name="xidx", bufs=2) as xpool2,
            tc.tile_pool(name="cout", bufs=2) as opool,
        ):
            xt_sb = xpool.tile([128, XS, KT, BL], f16)
            nc.sync.dma_start(out=xt_sb[:], in_=xt_r)
            ident = kpool.tile([128, 128], f16)
            make_identity(nc, ident[:])
            identf = kpool.tile([128, 128], f32)
            make_identity(nc, identf[:])
            gid = kpool.tile([128, NG], i32)
            nc.gpsimd.iota(gid[:], pattern=[[1, NG]], base=0, channel_multiplier=0)
            tagi = kpool.tile([128, GE], i32)
            nc.gpsimd.iota(tagi[:], pattern=[[1, GE]], base=0,
                           channel_multiplier=0)

            def pair_phase_a(t0):
                # two row-tiles share each W chunk load (halves W traffic)
                Gs = [gpool.tile([128, NG], f16, name=f"G{_i}") for _i in range(2)]
                preDs = [dpool.tile([128, FF], f32, name=f"preD{_i}") for _i in range(2)]
                abufs = [None, None]
                for fc in range(NFC):
                    w = wpool.tile([128, WS, KT, FC], f16, name="w")
                    nc.sync.dma_start(out=w[:], in_=wencT_r[:, fc, :, :, :])
                    # interleave the pair's tiles: consecutive matmuls alternate
                    # PSUM bank and weights so fill/drain pipelines
                    pss = [apsum.tile([128, FC], f32, name="ps") for _ in range(2)]
                    for gi, (xs_, ws_) in enumerate(PASSES):
                        for k in range(KT):
                            for ti in range(2):
                                nc.tensor.matmul(
                                    pss[ti][:], lhsT=xt_sb[:, xs_, k, ts(t0 + ti, 128)],
                                    rhs=w[:, ws_, k, :],
                                    start=(gi == 0 and k == 0),
                                    stop=(gi == len(PASSES) - 1 and k == KT - 1),
                                )
                    for ti in range(2):
                        ps = pss[ti]
                        # group maxima straight from PSUM (pre-relu; sign-safe)
                        psg = ps[:, :].rearrange("p (g e) -> p g e", e=GE)
                        nc.vector.reduce_max(
                            out=Gs[ti][:, ds(fc * GPC, GPC)], in_=psg,
                            axis=mybir.AxisListType.X)
                        # relu into fp32 staging block; spill every CB chunks
                        cb = fc % CB
                        if cb == 0:
                            abufs[ti] = apool.tile([128, CB * FC], f32, name=f"ab{ti}")
                        nc.scalar.activation(abufs[ti][:, ds(cb * FC, FC)], ps[:],
                                             Act.Relu)
                        if cb == CB - 1:
                            nc.scalar.dma_start(
                                out=preDs[ti][:, ds((fc - cb) * FC, CB * FC)],
                                in_=abufs[ti][:])
                return Gs, preDs

            def tile_body(t, G, preD):
                if ABLATE == "a":
                    return
                rows = ts(t, 128)
                preD_g = preD[:, :].rearrange("b (g e) -> (b g) e", e=GE)
                # ---------- Phase B ----------
                # pack group maxima: (fp16 bits << 16) | group id
                gpk = ppool.tile([128, NG], u32, name="gpk")
                nc.vector.tensor_copy(gpk[:], G[:, :].bitcast(u16))
                nc.vector.tensor_scalar(out=gpk[:], in0=gpk[:], scalar1=16,
                                        scalar2=None, op0=Alu.logical_shift_left)
                nc.vector.tensor_tensor(out=gpk[:], in0=gpk[:], in1=gid[:, :].bitcast(u32),
                                        op=Alu.bitwise_or)
                gpkf = gpk[:, :].bitcast(f32)
                gtop = spool.tile([128, NP], f32, name="gtop")
                for r in range(NP // 8):
                    mv = gtop[:, ds(r * 8, 8)]
                    nc.vector.max(out=mv, in_=gpkf)
                    if r < NP // 8 - 1:
                        nc.vector.match_replace(out=gpkf, in_to_replace=mv,
                                                in_values=gpkf, imm_value=0.0)
                gsel = spool.tile([128, NP], u32, name="gsel")
                nc.vector.tensor_scalar(out=gsel[:], in0=gtop[:, :].bitcast(u32),
                                        scalar1=0xFFFF, scalar2=None, op0=Alu.bitwise_and)
                goff = spool.tile([128, NP], i32, name="goff")
                nc.gpsimd.iota(goff[:], pattern=[[0, NP]], base=0,
                               channel_multiplier=NG)
                nc.vector.tensor_tensor(out=goff[:], in0=goff[:], in1=gsel[:, :].bitcast(i32),
                                        op=Alu.add)
                if ABLATE == "bg":
                    return
                cand = cpool.tile([128, NP, GE], f32, name="cand")
                for kk in range(NP):
                    nc.gpsimd.indirect_dma_start(
                        out=cand[:, kk, :], out_offset=None, in_=preD_g,
                        in_offset=bass.IndirectOffsetOnAxis(ap=goff[:, kk:kk + 1], axis=0),
                    )
                cand2 = cand[:, :, :].rearrange("p n e -> p (n e)")
                # feature tags: gsel*GE + elem
                gsel_b = gsel[:, :].rearrange("p (n o) -> p n o", o=1).to_broadcast([128, NP, GE])
                tagi_b = tagi[:, :].rearrange("p (o e) -> p o e", o=1).to_broadcast(
                    [128, NP, GE]).bitcast(u32)
                tagm = ppool.tile([128, NP, GE], u32, name="tagm")
                nc.vector.tensor_scalar(out=tagm[:], in0=gsel_b, scalar1=GE,
                                        scalar2=None, op0=Alu.mult)
                nc.vector.tensor_tensor(out=tagm[:], in0=tagm[:], in1=tagi_b,
                                        op=Alu.add)
                # pack candidates: (fp16 bits << 16) | tag
                cbf = cpool.tile([128, NP * GE], f16, name="cbf")
                nc.vector.tensor_copy(cbf[:], cand2)
                cpk = cpool.tile([128, NP * GE], u32, name="cpk")
                nc.vector.tensor_copy(cpk[:], cbf[:, :].bitcast(u16))
                nc.vector.tensor_scalar(out=cpk[:], in0=cpk[:], scalar1=16,
                                        scalar2=None, op0=Alu.logical_shift_left)
                tagm2 = tagm[:, :, :].rearrange("p n e -> p (n e)")
                nc.vector.tensor_tensor(out=cpk[:], in0=cpk[:], in1=tagm2,
                                        op=Alu.bitwise_or)
                # exact t* from candidate values (destructive rounds on a copy)
                vr = cpool.tile([128, NP * GE], f32, name="vr")
                nc.vector.tensor_copy(vr[:], cand2)
                mvf = None
                for r in range(K // 8):
                    mvf = spool.tile([128, 8], f32, name="mvf")
                    nc.vector.max(out=mvf[:], in_=vr[:])
                    if r < K // 8 - 1:
                        nc.vector.match_replace(out=vr[:], in_to_replace=mvf[:],
                                                in_values=vr[:], imm_value=0.0)
                tstar = spool.tile([128, 1], f32, name="tstar")
                nc.vector.tensor_copy(tstar[:], mvf[:, 7:8])
                # mask packed array to the exact selection, then extract pairs
                cpkf = cpk[:, :].bitcast(f32)
                nc.vector.scalar_tensor_tensor(
                    out=cpkf, in0=cand2, scalar=tstar[:], in1=cpkf,
                    op0=Alu.is_ge, op1=Alu.mult,
                )
                pk = spool.tile([128, K], f32, name="pk")
                for r in range(K // 8):
                    mv = pk[:, ds(r * 8, 8)]
                    nc.vector.max(out=mv, in_=cpkf)
                    if r < K // 8 - 1:
                        nc.vector.match_replace(out=cpkf, in_to_replace=mv,
                                                in_values=cpkf, imm_value=0.0)
                fsel = spool.tile([128, K], u32, name="fsel")
                nc.vector.tensor_scalar(out=fsel[:], in0=pk[:, :].bitcast(u32),
                                        scalar1=0xFFFF, scalar2=None, op0=Alu.bitwise_and)
                wbits = spool.tile([128, K], u32, name="wbits")
                nc.vector.tensor_scalar(out=wbits[:], in0=pk[:, :].bitcast(u32),
                                        scalar1=16, scalar2=None,
                                        op0=Alu.logical_shift_right)
                wnarrow = spool.tile([128, K], u16, name="wnarrow")
                nc.vector.tensor_copy(wnarrow[:], wbits[:])
                wsel = spool.tile([128, K], f32, name="wsel")
                nc.vector.tensor_copy(wsel[:], wnarrow[:, :].bitcast(f16))
                if ABLATE == "b":
                    return

                # ---------- Phase C ----------
                ND2 = max(1, DO // 512)
                DW = DO // ND2
                psD = [dpsum.tile([128, DW], f32, name=f"psD{_h}") for _h in range(ND2)]
                nmm = [0] * ND2
                for blk in range(K // PB):
                    wg = wgpool.tile([128, PB, DO], f16, name="wg")
                    for j in range(PB):
                        nc.gpsimd.indirect_dma_start(
                            out=wg[:, j, :], out_offset=None, in_=wdecT[:, :],
                            in_offset=bass.IndirectOffsetOnAxis(
                                ap=fsel[:, blk * PB + j:blk * PB + j + 1], axis=0),
                        )
                    dgs = []
                    for j in range(PB):
                        k = blk * PB + j
                        dg = dgpool.tile([128, 128], f16, name="dg")
                        nc.vector.tensor_scalar(out=dg[:], in0=ident[:],
                                                scalar1=wsel[:, k:k + 1], scalar2=None,
                                                op0=Alu.mult)
                        dgs.append(dg)
                    # alternate bank AND weights on consecutive matmuls
                    for p2 in range(ND2):
                        for j in range(PB):
                            h = (j + p2) % ND2
                            nc.tensor.matmul(psD[h][:], lhsT=dgs[j][:],
                                             rhs=wg[:, j, ds(h * DW, DW)],
                                             start=(nmm[h] == 0),
                                             stop=(nmm[h] == K - 1))
                            nmm[h] += 1
                co = opool.tile([128, DO], f32, name="co")
                for h in range(ND2):
                    nc.vector.tensor_copy(co[:, ds(h * DW, DW)], psD[h][:])
                nc.scalar.dma_start(out=out.ap()[rows, :], in_=co[:])

            def full_pass(stages=False):
                # emit ALL phase-A pairs first so the PE instruction stream
                # never waits on a top-K chain before starting the next pair
                pas = []
                for t0 in range(0, NT, 2):
                    pas.append(pair_phase_a(t0))
                    if stages:
                        tc.stage_boundary()
                for pi, t0 in enumerate(range(0, NT, 2)):
                    Gs, preDs = pas[pi]
                    tile_body(t0, Gs[0], preDs[0])
                    tile_body(t0 + 1, Gs[1], preDs[1])
                    if stages and pi == 0:
                        tc.stage_boundary()

            if reps > 1:
                with tc.For_i(0, reps, 1, staggered_reset=True):
                    full_pass(stages=True)
            else:
                full_pass()

    nc.compile()
    return nc


def get_kernel(DX, reps=1):
    key = (DX, reps, NSPLIT)
    if key not in _CACHE:
        _CACHE[key] = build(DX, D, F, B // N_CORES, N_CORES, reps=reps)
    return _CACHE[key]


def prep_in_maps(x, W_enc, b_enc, W_dec, b_dec):
    BL = B // N_CORES
    xs = (x - b_dec).astype(np.float32)
    wencT = np.ascontiguousarray(W_enc.T.astype(np.float32))   # [D, F]
    if np.any(b_enc):
        # fold b_enc in as one extra 128-row contraction tile
        DX = D + 128
        xa = np.zeros((B, DX), np.float32)
        xa[:, :D] = xs
        xa[:, D] = 1.0
        wa = np.zeros((DX, F), np.float32)
        wa[:D] = wencT
        wa[D] = b_enc
        xs, wencT = xa, wa
    else:
        DX = D
    xst = np.ascontiguousarray(xs.T)                            # [DX, B]
    wdecT = np.ascontiguousarray(W_dec.T).astype(np.float16)    # [F, D]
    wh = wencT.astype(np.float16)
    xh = xst.astype(np.float16)
    if NSPLIT == 3:
        # fp16 3-split operands (exact to ~2^-22): hi, scaled-lo, residual
        wls = ((wencT - wh.astype(np.float32)) * 2.0 ** 11).astype(np.float16)
        wenc = np.stack([wh, wls])                              # [2, DX, F]
        xhs = (xh.astype(np.float32) * 2.0 ** -11).astype(np.float16)
        xl = (xst - xh.astype(np.float32)).astype(np.float16)
        xstk = np.stack([xh, xhs, xl])                          # [3, DX, B]
    else:
        # 2-split: x exact (hi+lo), W fp16 only
        wenc = wh[None]                                         # [1, DX, F]
        xl = (xst - xh.astype(np.float32)).astype(np.float16)
        xstk = np.stack([xh, xl])                               # [2, DX, B]
    # block W per chunk: [WS, DX, F] -> [NFC, 128, WS, KT, FC]
    WS, FC = wenc.shape[0], 512
    KT, NFC = DX // 128, F // FC
    wenc = np.ascontiguousarray(
        wenc.reshape(WS, KT, 128, NFC, FC).transpose(3, 2, 0, 1, 4))
    in_maps = [{
        "xt": np.ascontiguousarray(xstk[:, :, c * BL:(c + 1) * BL]),
        "wencT": wenc,
        "wdecT": wdecT,
    } for c in range(N_CORES)]
    return in_maps, DX


def kernel(x, W_enc, b_enc, W_dec, b_dec):
    x = np.asarray(x, np.float32)
    W_enc = np.asarray(W_enc, np.float32)
    b_enc = np.asarray(b_enc, np.float32)
    W_dec = np.asarray(W_dec, np.float32)
    b_dec = np.asarray(b_dec, np.float32)
    in_maps, DX = prep_in_maps(x, W_enc, b_enc, W_dec, b_dec)
    nc = get_kernel(DX)
    res = run_bass_kernel_spmd(nc, in_maps, list(range(N_CORES)))
    y = np.concatenate([res.results[c]["out"] for c in range(N_CORES)], axis=0)
    return (y + b_dec).astype(np.float32)


# revision 47
# speedup vs baseline: 1.4057x; 1.4057x over previous
"""TopK autoencoder (B=4096, D=1024, F=32768, K=64) on 8 Trainium2 NeuronCores.

Strategy: data-parallel over batch (512 rows/core). Per core, per 128-row tile:
  A) fp16-split encoder matmul (PE) accumulated in PSUM fp32; relu on ACT into
     an fp32 staging buffer spilled to DRAM in 1MB blocks; per-group (32) maxima
     reduced on DVE straight from PSUM (pre-relu max == relu'd max for the
     groups that matter).
  B) exact top-K: group maxima packed as (fp16-bits << 16 | group-id); top-NP
     groups via max8/match_replace rounds; ONE batched indirect DMA gathers all
     NP*32 fp32 candidates; candidates packed with feature tags; exact K-th
     value from destructive max rounds; threshold-mask then extract (value,
     index) pairs.
  C) decode: batched indirect gather of selected W_dec rows (fp16), accumulate
     w_k * row_k on the PE via diagonal-matrix matmuls into PSUM.
b_dec is handled exactly on the host (x - b_dec, + b_dec at the end); a
nonzero b_enc is folded in as an extra contraction tile (zero here).
"""
import sys
sys.path.insert(0, '/opt/trn_rl_repo')
import numpy as np
import concourse.bass as bass
import concourse.mybir as mybir
from concourse import bacc
from concourse.bass import ts, ds
from concourse.tile import TileContext
from concourse.masks import make_identity
from concourse.bass_utils import run_bass_kernel_spmd

f32 = mybir.dt.float32
f16 = mybir.dt.float16
bf16 = mybir.dt.bfloat16
u16 = mybir.dt.uint16
u32 = mybir.dt.uint32
i32 = mybir.dt.int32
i16 = mybir.dt.int16
Alu = mybir.AluOpType
Act = mybir.ActivationFunctionType

B, D, F, K = 4096, 1024, 32768, 64
N_CORES = 8
GE = 32      # group size
NP = 72      # candidate groups per row
PB = 8       # decode gather block
NSPLIT = 2   # matmul passes: 3 = exact-fp32 selection; 2 = x-split only

_CACHE = {}
ABLATE = None   # None | "a" | "bg" | "b"  (timing experiments only)


def build(DX, DO, FF, BL, n_cores=N_CORES, reps=1, nsplit=NSPLIT):
    """Per-core kernel. DX: contraction dim (may include bias tile), DO: output dim."""
    KT = DX // 128
    NT = BL // 128
    FC = 512
    NFC = FF // FC
    NG = FF // GE
    GPC = FC // GE    # groups per chunk
    CB = 4            # chunks per spill block
    WS = 2 if nsplit == 3 else 1   # number of W split planes
    XS = nsplit                    # number of x planes
    # pass structure: (x plane, w plane)
    if nsplit == 3:
        PASSES = ((0, 0), (1, 1), (2, 0))
    else:
        PASSES = ((0, 0), (1, 0))

    nc = bacc.Bacc("TRN2", target_bir_lowering=False, debug=False, num_devices=n_cores)
    xt = nc.dram_tensor("xt", [XS, DX, BL], f16, kind="ExternalInput")
    # W blocked per chunk: [NFC, 128, WS, KT, FC] so each chunk load is
    # contiguous 16KB per partition (line-rate DMA)
    wencT = nc.dram_tensor("wencT", [NFC, 128, WS, KT, FC], f16, kind="ExternalInput")
    wdecT = nc.dram_tensor("wdecT", [FF, DO], f16, kind="ExternalInput")
    out = nc.dram_tensor("out", [BL, DO], f32, kind="ExternalOutput")

    wencT_r = wencT.ap().rearrange("n p s k f -> p n s k f")
    xt_r = xt.ap().rearrange("s (k p) b -> p s k b", p=128)

    with TileContext(nc) as tc:
        with (
            tc.tile_pool(name="dram", bufs=4, space="DRAM") as dpool,
            tc.tile_pool(name="xt_sb", bufs=1) as xpool,
            tc.tile_pool(name="const", bufs=1) as kpool,
            tc.tile_pool(name="wenc", bufs=3) as wpool,
            tc.tile_pool(name="apsum", bufs=6, space="PSUM") as apsum,
            tc.tile_pool(name="abuf", bufs=2) as apool,
            tc.tile_pool(name="dg", bufs=8) as dgpool,
            tc.tile_pool(name="gbuf", bufs=4) as gpool,
            tc.tile_pool(name="cand", bufs=1) as cpool,
            tc.tile_pool(name="pack", bufs=1) as ppool,
            tc.tile_pool(name="small", bufs=2) as spool,
            tc.tile_pool(name="wdecg", bufs=2) as wgpool,
            tc.tile_pool(name="dpsum", bufs=1, space="PSUM") as dpsum,
            tc.tile_pool(name="cout", bufs=2) as opool,
        ):
            xt_sb = xpool.tile([128, XS, KT, BL], f16)
            nc.sync.dma_start(out=xt_sb[:], in_=xt_r)
            ident = kpool.tile([128, 128], f16)
            make_identity(nc, ident[:])
            gid = kpool.tile([128, NG], i32)
            nc.gpsimd.iota(gid[:], pattern=[[1, NG]], base=0, channel_multiplier=0)
            tagi = kpool.tile([128, GE], i32)
            nc.gpsimd.iota(tagi[:], pattern=[[1, GE]], base=0,
                           channel_multiplier=0)

            def pair_phase_a(t0):
                # two row-tiles share each W chunk load (halves W traffic)
                Gs = [gpool.tile([128, NG], f16, name=f"G{_i}") for _i in range(2)]
                preDs = [dpool.tile([128, FF], f32, name=f"preD{_i}") for _i in range(2)]
                abufs = [None, None]
                for fc in range(NFC):
                    w = wpool.tile([128, WS, KT, FC], f16, name="w")
                    nc.sync.dma_start(out=w[:], in_=wencT_r[:, fc, :, :, :])
                    # interleave the pair's tiles: consecutive matmuls alternate
                    # PSUM bank and weights so fill/drain pipelines
                    pss = [apsum.tile([128, FC], f32, name="ps") for _ in range(2)]
                    for gi, (xs_, ws_) in enumerate(PASSES):
                        for k in range(KT):
                            for ti in range(2):
                                nc.tensor.matmul(
                                    pss[ti][:], lhsT=xt_sb[:, xs_, k, ts(t0 + ti, 128)],
                                    rhs=w[:, ws_, k, :],
                                    start=(gi == 0 and k == 0),
                                    stop=(gi == len(PASSES) - 1 and k == KT - 1),
                                )
                    for ti in range(2):
                        ps = pss[ti]
                        # group maxima straight from PSUM (pre-relu; sign-safe)
                        psg = ps[:, :].rearrange("p (g e) -> p g e", e=GE)
                        nc.vector.reduce_max(
                            out=Gs[ti][:, ds(fc * GPC, GPC)], in_=psg,
                            axis=mybir.AxisListType.X)
                        # relu into fp32 staging block; spill every CB chunks
                        cb = fc % CB
                        if cb == 0:
                            abufs[ti] = apool.tile([128, CB * FC], f32, name=f"ab{ti}")
                        nc.scalar.activation(abufs[ti][:, ds(cb * FC, FC)], ps[:],
                                             Act.Relu)
                        if cb == CB - 1:
                            nc.scalar.dma_start(
                                out=preDs[ti][:, ds((fc - cb) * FC, CB * FC)],
                                in_=abufs[ti][:])
                return Gs, preDs

            def tile_body(t, G, preD):
                if ABLATE == "a":
                    return
                rows = ts(t, 128)
                preD_g = preD[:, :].rearrange("b (g e) -> (b g) e", e=GE)
                # ---------- Phase B ----------
                # pack group maxima: (fp16 bits << 16) | group id
                gpk = ppool.tile([128, NG], u32, name="gpk")
                nc.vector.tensor_copy(gpk[:], G[:, :].bitcast(u16))
                nc.vector.tensor_scalar(out=gpk[:], in0=gpk[:], scalar1=16,
                                        scalar2=None, op0=Alu.logical_shift_left)
                nc.vector.tensor_tensor(out=gpk[:], in0=gpk[:], in1=gid[:, :].bitcast(u32),
                                        op=Alu.bitwise_or)
                gpkf = gpk[:, :].bitcast(f32)
                gtop = spool.tile([128, NP], f32, name="gtop")
                for r in range(NP // 8):
                    mv = gtop[:, ds(r * 8, 8)]
                    nc.vector.max(out=mv, in_=gpkf)
                    if r < NP // 8 - 1:
                        nc.vector.match_replace(out=gpkf, in_to_replace=mv,
                                                in_values=gpkf, imm_value=0.0)
                gsel = spool.tile([128, NP], u32, name="gsel")
                nc.vector.tensor_scalar(out=gsel[:], in0=gtop[:, :].bitcast(u32),
                                        scalar1=0xFFFF, scalar2=None, op0=Alu.bitwise_and)
                goff = spool.tile([128, NP], i32, name="goff")
                nc.gpsimd.iota(goff[:], pattern=[[0, NP]], base=0,
                               channel_multiplier=NG)
                nc.vector.tensor_tensor(out=goff[:], in0=goff[:], in1=gsel[:, :].bitcast(i32),
                                        op=Alu.add)
                if ABLATE == "bg":
                    return
                cand = cpool.tile([128, NP, GE], f32, name="cand")
                for kk in range(NP):
                    nc.gpsimd.indirect_dma_start(
                        out=cand[:, kk, :], out_offset=None, in_=preD_g,
                        in_offset=bass.IndirectOffsetOnAxis(ap=goff[:, kk:kk + 1], axis=0),
                    )
                cand2 = cand[:, :, :].rearrange("p n e -> p (n e)")
                # feature tags: gsel*GE + elem
                gsel_b = gsel[:, :].rearrange("p (n o) -> p n o", o=1).to_broadcast([128, NP, GE])
                tagi_b = tagi[:, :].rearrange("p (o e) -> p o e", o=1).to_broadcast(
                    [128, NP, GE]).bitcast(u32)
                tagm = ppool.tile([128, NP, GE], u32, name="tagm")
                nc.vector.tensor_scalar(out=tagm[:], in0=gsel_b, scalar1=GE,
                                        scalar2=None, op0=Alu.mult)
                nc.vector.tensor_tensor(out=tagm[:], in0=tagm[:], in1=tagi_b,
                                        op=Alu.add)
                # pack candidates: (fp16 bits << 16) | tag
                cbf = cpool.tile([128, NP * GE], f16, name="cbf")
                nc.vector.tensor_copy(cbf[:], cand2)
                cpk = cpool.tile([128, NP * GE], u32, name="cpk")
                nc.vector.tensor_copy(cpk[:], cbf[:, :].bitcast(u16))
                nc.vector.tensor_scalar(out=cpk[:], in0=cpk[:], scalar1=16,
                                        scalar2=None, op0=Alu.logical_shift_left)
                tagm2 = tagm[:, :, :].rearrange("p n e -> p (n e)")
                nc.vector.tensor_tensor(out=cpk[:], in0=cpk[:], in1=tagm2,
                                        op=Alu.bitwise_or)
                # exact t* from candidate values (destructive rounds on a copy)
                vr = cpool.tile([128, NP * GE], f32, name="vr")
                nc.vector.tensor_copy(vr[:], cand2)
                mvf = None
                for r in range(K // 8):
                    mvf = spool.tile([128, 8], f32, name="mvf")
                    nc.vector.max(out=mvf[:], in_=vr[:])
                    if r < K // 8 - 1:
                        nc.vector.match_replace(out=vr[:], in_to_replace=mvf[:],
                                                in_values=vr[:], imm_value=0.0)
                tstar = spool.tile([128, 1], f32, name="tstar")
                nc.vector.tensor_copy(tstar[:], mvf[:, 7:8])
                # mask packed array to the exact selection, then extract pairs
                cpkf = cpk[:, :].bitcast(f32)
                nc.vector.scalar_tensor_tensor(
                    out=cpkf, in0=cand2, scalar=tstar[:], in1=cpkf,
                    op0=Alu.is_ge, op1=Alu.mult,
                )
                pk = spool.tile([128, K], f32, name="pk")
                for r in range(K // 8):
                    mv = pk[:, ds(r * 8, 8)]
                    nc.vector.max(out=mv, in_=cpkf)
                    if r < K // 8 - 1:
                        nc.vector.match_replace(out=cpkf, in_to_replace=mv,
                                                in_values=cpkf, imm_value=0.0)
                fsel = spool.tile([128, K], u32, name="fsel")
                nc.vector.tensor_scalar(out=fsel[:], in0=pk[:, :].bitcast(u32),
                                        scalar1=0xFFFF, scalar2=None, op0=Alu.bitwise_and)
                wbits = spool.tile([128, K], u32, name="wbits")
                nc.vector.tensor_scalar(out=wbits[:], in0=pk[:, :].bitcast(u32),
                                        scalar1=16, scalar2=None,
                                        op0=Alu.logical_shift_right)
                wnarrow = spool.tile([128, K], u16, name="wnarrow")
                nc.vector.tensor_copy(wnarrow[:], wbits[:])
                wsel = spool.tile([128, K], f32, name="wsel")
                nc.vector.tensor_copy(wsel[:], wnarrow[:, :].bitcast(f16))
                if ABLATE == "b":
                    return

                # ---------- Phase C ----------
                ND2 = max(1, DO // 512)
                DW = DO // ND2
                psD = [dpsum.tile([128, DW], f32, name=f"psD{_h}") for _h in range(ND2)]
                nmm = [0] * ND2
                for blk in range(K // PB):
                    wg = wgpool.tile([128, PB, DO], f16, name="wg")
                    for j in range(PB):
                        nc.gpsimd.indirect_dma_start(
                            out=wg[:, j, :], out_offset=None, in_=wdecT[:, :],
                            in_offset=bass.IndirectOffsetOnAxis(
                                ap=fsel[:, blk * PB + j:blk * PB + j + 1], axis=0),
                        )
                    dgs = []
                    for j in range(PB):
                        k = blk * PB + j
                        dg = dgpool.tile([128, 128], f16, name="dg")
                        nc.vector.tensor_scalar(out=dg[:], in0=ident[:],
                                                scalar1=wsel[:, k:k + 1], scalar2=None,
                                                op0=Alu.mult)
                        dgs.append(dg)
                    # alternate bank AND weights on consecutive matmuls
                    for p2 in range(ND2):
                        for j in range(PB):
                            h = (j + p2) % ND2
                            nc.tensor.matmul(psD[h][:], lhsT=dgs[j][:],
                                             rhs=wg[:, j, ds(h * DW, DW)],
                                             start=(nmm[h] == 0),
                                             stop=(nmm[h] == K - 1))
                            nmm[h] += 1
                co = opool.tile([128, DO], f32, name="co")
                for h in range(ND2):
                    nc.vector.tensor_copy(co[:, ds(h * DW, DW)], psD[h][:])
                nc.scalar.dma_start(out=out.ap()[rows, :], in_=co[:])

            def full_pass():
                # emit ALL phase-A pairs first so the PE instruction stream
                # never waits on a top-K chain before starting the next pair
                pas = [pair_phase_a(t0) for t0 in range(0, NT, 2)]
                for pi, t0 in enumerate(range(0, NT, 2)):
                    Gs, preDs = pas[pi]
                    tile_body(t0, Gs[0], preDs[0])
                    tile_body(t0 + 1, Gs[1], preDs[1])

            if reps > 1:
                with tc.For_i(0, reps, 1):
                    full_pass()
            else:
                full_pass()

    nc.compile()
    return nc


def get_kernel(DX, reps=1):
    key = (DX, reps, NSPLIT)
    if key not in _CACHE:
        _CACHE[key] = build(DX, D, F, B // N_CORES, N_CORES, reps=reps)
    return _CACHE[key]


def prep_in_maps(x, W_enc, b_enc, W_dec, b_dec):
    BL = B // N_CORES
    xs = (x - b_dec).astype(np.float32)
    wencT = np.ascontiguousarray(W_enc.T.astype(np.float32))   # [D, F]
    if np.any(b_enc):
        # fold b_enc in as one extra 128-row contraction tile
        DX = D + 128
        xa = np.zeros((B, DX), np.float32)
        xa[:, :D] = xs
        xa[:, D] = 1.0
        wa = np.zeros((DX, F), np.float32)
        wa[:D] = wencT
        wa[D] = b_enc
        xs, wencT = xa, wa
    else:
        DX = D
    xst = np.ascontiguousarray(xs.T)                            # [DX, B]
    wdecT = np.ascontiguousarray(W_dec.T).astype(np.float16)    # [F, D]
    wh = wencT.astype(np.float16)
    xh = xst.astype(np.float16)
    if NSPLIT == 3:
        # fp16 3-split operands (exact to ~2^-22): hi, scaled-lo, residual
        wls = ((wencT - wh.astype(np.float32)) * 2.0 ** 11).astype(np.float16)
        wenc = np.stack([wh, wls])                              # [2, DX, F]
        xhs = (xh.astype(np.float32) * 2.0 ** -11).astype(np.float16)
        xl = (xst - xh.astype(np.float32)).astype(np.float16)
        xstk = np.stack([xh, xhs, xl])                          # [3, DX, B]
    else:
        # 2-split: x exact (hi+lo), W fp16 only
        wenc = wh[None]                                         # [1, DX, F]
        xl = (xst - xh.astype(np.float32)).astype(np.float16)
        xstk = np.stack([xh, xl])                               # [2, DX, B]
    # block W per chunk: [WS, DX, F] -> [NFC, 128, WS, KT, FC]
    WS, FC = wenc.shape[0], 512
    KT, NFC = DX // 128, F // FC
    wenc = np.ascontiguousarray(
        wenc.reshape(WS, KT, 128, NFC, FC).transpose(3, 2, 0, 1, 4))
    in_maps = [{
        "xt": np.ascontiguousarray(xstk[:, :, c * BL:(c + 1) * BL]),
        "wencT": wenc,
        "wdecT": wdecT,
    } for c in range(N_CORES)]
    return in_maps, DX


def kernel(x, W_enc, b_enc, W_dec, b_dec):
    x = np.asarray(x, np.float32)
    W_enc = np.asarray(W_enc, np.float32)
    b_enc = np.asarray(b_enc, np.float32)
    W_dec = np.asarray(W_dec, np.float32)
    b_dec = np.asarray(b_dec, np.float32)
    in_maps, DX = prep_in_maps(x, W_enc, b_enc, W_dec, b_dec)
    nc = get_kernel(DX)
    res = run_bass_kernel_spmd(nc, in_maps, list(range(N_CORES)))
    y = np.concatenate([res.results[c]["out"] for c in range(N_CORES)], axis=0)
    return (y + b_dec).astype(np.float32)
